# revision 3
# baseline (speedup 1.0000x reference)
"""Trainium2 Bass kernel for nn_ArgumentLocalLogits.

Math (uniform segments, BS=16, CTX_PER=1024, ARGS_PER=32):
  keys   = ctx_values @ W + b                    [n_ctx, 128]
  logits[1024*a + j] = dot(arg_values[a], keys[1024*seg(a) + j])
  rows[p] = p // 1024

Sharding: 2 proof states (segments) per core across 8 cores. Each core
gets its ctx shard pre-transposed on host (d_model on partitions) so the
on-chip work is pure matmul:
  K^T[dk, ctx] = sum_k W_k^T @ CT_k  (+b)        PSUM-accumulated over 4
                                                 d_model chunks of 128
  logits_s[arg, ctx] = (A_s^T)^T @ K_s^T         per segment
"""

import numpy as np

BS = 16
CTX_PER = 1024
ARGS_PER = 32
KEY_DIM = 128
D_MODEL = 512
N_CORES = 8
SEG_PER_CORE = BS // N_CORES          # 2
CTX_SHARD = SEG_PER_CORE * CTX_PER    # 2048
ARG_SHARD = SEG_PER_CORE * ARGS_PER   # 64
KCH = D_MODEL // 128                  # 4 contraction chunks
CTX_CHUNK = 512                       # ctx pipeline chunk (matmul N)
N_CHUNKS = CTX_SHARD // CTX_CHUNK     # 4

_BUILT = {}


def _build_nc(mm_dtype_name: str):
    import concourse.tile as tile
    from concourse import bacc, mybir

    mm_dt = getattr(mybir.dt, mm_dtype_name)
    f32 = mybir.dt.float32

    nc = bacc.Bacc(None, target_bir_lowering=False)
    ct = nc.dram_tensor("ct", [D_MODEL, CTX_SHARD], mm_dt, kind="ExternalInput")
    at = nc.dram_tensor("at", [KEY_DIM, ARG_SHARD], mm_dt, kind="ExternalInput")
    w = nc.dram_tensor("w", [D_MODEL, KEY_DIM], mm_dt, kind="ExternalInput")
    b = nc.dram_tensor("b", [KEY_DIM, 1], f32, kind="ExternalInput")
    out = nc.dram_tensor("out", [ARG_SHARD, CTX_PER], f32, kind="ExternalOutput")

    # [512, 2048] -> [128, 4(k), 2048] so chunk k sits on partitions
    ct_v = ct.rearrange("(k p) c -> p k c", k=KCH)
    w_v = w.rearrange("(k p) d -> p k d", k=KCH)

    with tile.TileContext(nc) as tc:
        with (
            tc.tile_pool(name="consts", bufs=1) as consts,
            tc.tile_pool(name="ctp", bufs=N_CHUNKS) as ctp,
            tc.tile_pool(name="kts", bufs=1) as kts,
            tc.tile_pool(name="lgs", bufs=1) as lgs,
            tc.tile_pool(name="ktp", bufs=N_CHUNKS, space="PSUM") as ktp,
            tc.tile_pool(name="lgp", bufs=2, space="PSUM") as lgp,
        ):
            wt = consts.tile([128, KCH, KEY_DIM], mm_dt)
            nc.sync.dma_start(wt[:], w_v[:])
            att = consts.tile([KEY_DIM, ARG_SHARD], mm_dt)
            nc.sync.dma_start(att[:], at[:])
            bt = consts.tile([KEY_DIM, 1], f32)
            nc.sync.dma_start(bt[:], b[:])

            kt_sb = kts.tile([KEY_DIM, CTX_SHARD], mm_dt)
            lg_sb = lgs.tile([ARG_SHARD, CTX_PER], f32)

            for j in range(N_CHUNKS):
                cs = slice(j * CTX_CHUNK, (j + 1) * CTX_CHUNK)
                ctt = ctp.tile([128, KCH, CTX_CHUNK], mm_dt)
                nc.sync.dma_start(ctt[:], ct_v[:, :, cs])

                kt_ps = ktp.tile([KEY_DIM, CTX_CHUNK], f32)
                for k in range(KCH):
                    nc.tensor.matmul(
                        kt_ps[:],
                        wt[:, k, :],
                        ctt[:, k, :],
                        start=(k == 0),
                        stop=(k == KCH - 1),
                    )
                # PSUM -> SBUF with bias add (b is per-partition here)
                nc.vector.tensor_scalar_add(kt_sb[:, cs], kt_ps[:], bt[:])

                # each segment covers CTX_PER/CTX_CHUNK consecutive chunks
                s = (j * CTX_CHUNK) // CTX_PER
                jj = j % (CTX_PER // CTX_CHUNK)
                lg_ps = lgp.tile([ARGS_PER, CTX_CHUNK], f32)
                nc.tensor.matmul(
                    lg_ps[:],
                    att[:, s * ARGS_PER : (s + 1) * ARGS_PER],
                    kt_sb[:, cs],
                    start=True,
                    stop=True,
                )
                oslice = (
                    slice(s * ARGS_PER, (s + 1) * ARGS_PER),
                    slice(jj * CTX_CHUNK, (jj + 1) * CTX_CHUNK),
                )
                nc.scalar.activation(
                    lg_sb[oslice], lg_ps[:], mybir.ActivationFunctionType.Copy
                )
                if jj == CTX_PER // CTX_CHUNK - 1:
                    nc.sync.dma_start(
                        out[s * ARGS_PER : (s + 1) * ARGS_PER, :],
                        lg_sb[s * ARGS_PER : (s + 1) * ARGS_PER, :],
                    )
    nc.finalize()
    return nc


def _get_nc(mm_dtype_name: str):
    if mm_dtype_name not in _BUILT:
        _BUILT[mm_dtype_name] = _build_nc(mm_dtype_name)
    return _BUILT[mm_dtype_name]


def _uniform_structure(bs, arg_ids, ctx_ids):
    if bs != BS or arg_ids.shape[0] != BS * ARGS_PER or ctx_ids.shape[0] != BS * CTX_PER:
        return False
    if not np.array_equal(np.asarray(arg_ids), np.repeat(np.arange(BS, dtype=np.int32), ARGS_PER)):
        return False
    if not np.array_equal(np.asarray(ctx_ids), np.repeat(np.arange(BS, dtype=np.int32), CTX_PER)):
        return False
    return True


def _reference_host(bs, arg_ids, ctx_ids, arg_values, ctx_values, W, b):
    """Numpy mirror of the oracle — correctness fallback for non-uniform ids."""
    n_args = arg_ids.shape[0]
    n_ctx = ctx_ids.shape[0]
    P = n_args * (n_ctx // bs)
    ctx_lens = np.bincount(ctx_ids, minlength=bs)
    arg_ctx_lens = ctx_lens[arg_ids]
    arg_ends = np.cumsum(arg_ctx_lens)
    arg_starts = arg_ends - arg_ctx_lens
    pos = np.arange(P, dtype=arg_ends.dtype)
    rows = np.searchsorted(arg_ends, pos, side="right")
    rows_c = np.clip(rows, 0, n_args - 1)
    offs = pos - arg_starts[rows_c]
    ctx_starts = np.cumsum(ctx_lens) - ctx_lens
    cols = ctx_starts[arg_ids[rows_c]] + offs
    cols = np.clip(cols, 0, n_ctx - 1)
    keys_all = ctx_values @ W + b
    logits = np.einsum(
        "pd,pd->p", arg_values[rows_c], keys_all[cols], optimize=True
    ).astype(np.float32)
    return rows.astype(np.int32), logits


LAST_EXEC_NS = None


def _install_ntff_hook():
    """Test-only: register the NTFF profile hook if the image lacks it."""
    import sys, types
    try:
        from antenv.axon_hooks import get_axon_ntff_profile_hook  # noqa: F401
        return
    except ImportError:
        pass
    import antenv
    from trn_agent_boot.trn_boot import _ntff_profile_via_ctypes

    hooks_mod = types.ModuleType("antenv.axon_hooks")
    _hook = _ntff_profile_via_ctypes("/opt/axon/libaxon_pjrt.so")
    hooks_mod.get_axon_ntff_profile_hook = lambda: _hook
    hooks_mod.set_axon_ntff_profile_hook = lambda h: None
    sys.modules["antenv.axon_hooks"] = hooks_mod
    antenv.axon_hooks = hooks_mod


def kernel(bs, arg_ids, ctx_ids, arg_values, ctx_values, W, b,
           _mm_dtype="float32r", _profile=False):
    from concourse.bass_utils import run_bass_kernel_spmd

    bs = int(np.asarray(bs))
    arg_values = np.asarray(arg_values, dtype=np.float32)
    ctx_values = np.asarray(ctx_values, dtype=np.float32)
    W = np.asarray(W, dtype=np.float32)
    b = np.asarray(b, dtype=np.float32)

    if not _uniform_structure(bs, arg_ids, ctx_ids):
        return _reference_host(
            bs, np.asarray(arg_ids), np.asarray(ctx_ids), arg_values, ctx_values, W, b
        )

    nc = _get_nc(_mm_dtype)

    w_arr = np.ascontiguousarray(W)
    b_arr = np.ascontiguousarray(b.reshape(KEY_DIM, 1))
    in_maps = []
    for c in range(N_CORES):
        ct_c = np.ascontiguousarray(
            ctx_values[c * CTX_SHARD : (c + 1) * CTX_SHARD].T
        )
        at_c = np.ascontiguousarray(
            arg_values[c * ARG_SHARD : (c + 1) * ARG_SHARD].T
        )
        in_maps.append({"ct": ct_c, "at": at_c, "w": w_arr, "b": b_arr})

    kwargs = {}
    if _profile:
        _install_ntff_hook()
        kwargs["trace"] = True
    res = run_bass_kernel_spmd(nc, in_maps, core_ids=list(range(N_CORES)), **kwargs)
    global LAST_EXEC_NS
    LAST_EXEC_NS = res.exec_time_ns
    logits = np.concatenate(
        [np.asarray(res.results[c]["out"]).reshape(-1) for c in range(N_CORES)]
    )
    rows = np.repeat(np.arange(BS * ARGS_PER, dtype=np.int32), CTX_PER)
    return rows, logits


# revision 4
# speedup vs baseline: 1.0097x; 1.0097x over previous
"""Trainium2 Bass kernel for nn_ArgumentLocalLogits.

Math (uniform segments, BS=16, CTX_PER=1024, ARGS_PER=32):
  keys   = ctx_values @ W + b                    [n_ctx, 128]
  logits[1024*a + j] = dot(arg_values[a], keys[1024*seg(a) + j])
  rows[p] = p // 1024

Sharding: 2 proof states (segments) per core across 8 cores. Each core
gets its ctx shard pre-transposed/packed on host (d_model on partitions)
so the on-chip work is pure matmul:
  K^T[dk, ctx] = sum_k W_k^T @ CT_k  (+b)        PSUM-accumulated over 4
                                                 d_model chunks of 128
  logits_s[arg, ctx] = (A_s^T)^T @ K_s^T         per segment
"""

import numpy as np

BS = 16
CTX_PER = 1024
ARGS_PER = 32
KEY_DIM = 128
D_MODEL = 512
N_CORES = 8
SEG_PER_CORE = BS // N_CORES          # 2
CTX_SHARD = SEG_PER_CORE * CTX_PER    # 2048
ARG_SHARD = SEG_PER_CORE * ARGS_PER   # 64
KCH = D_MODEL // 128                  # 4 contraction chunks

# ctx pipeline chunks (must not cross the segment boundary at 1024);
# smaller tail chunks shrink the post-DMA critical path
CHUNKS = [(0, 512), (512, 512), (1024, 512), (1536, 256), (1792, 256)]

_BUILT = {}


def _build_nc(mm_dtype_name: str, with_bias: bool):
    import concourse.tile as tile
    from concourse import bacc, mybir

    mm_dt = getattr(mybir.dt, mm_dtype_name)
    f32 = mybir.dt.float32

    nc = bacc.Bacc(None, target_bir_lowering=False)
    # ct is packed on host as concat over chunks of [128, KCH, L] blocks
    ct = nc.dram_tensor("ct", [D_MODEL * CTX_SHARD], mm_dt, kind="ExternalInput")
    at = nc.dram_tensor("at", [KEY_DIM, ARG_SHARD], mm_dt, kind="ExternalInput")
    w = nc.dram_tensor("w", [128, KCH, KEY_DIM], mm_dt, kind="ExternalInput")
    if with_bias:
        b = nc.dram_tensor("b", [KEY_DIM, 1], f32, kind="ExternalInput")
    out = nc.dram_tensor("out", [ARG_SHARD, CTX_PER], f32, kind="ExternalOutput")

    with tile.TileContext(nc) as tc:
        with (
            tc.tile_pool(name="consts", bufs=1) as consts,
            tc.tile_pool(name="ctp", bufs=len(CHUNKS)) as ctp,
            tc.tile_pool(name="kts", bufs=1) as kts,
            tc.tile_pool(name="lgs", bufs=1) as lgs,
            tc.tile_pool(name="ktp", bufs=3, space="PSUM") as ktp,
            tc.tile_pool(name="lgp", bufs=2, space="PSUM") as lgp,
        ):
            wt = consts.tile([128, KCH, KEY_DIM], mm_dt)
            nc.scalar.dma_start(wt[:], w[:])
            att = consts.tile([KEY_DIM, ARG_SHARD], mm_dt)
            nc.scalar.dma_start(att[:], at[:])
            if with_bias:
                bt = consts.tile([KEY_DIM, 1], f32)
                nc.scalar.dma_start(bt[:], b[:])

            kt_sb = kts.tile([KEY_DIM, CTX_SHARD], mm_dt)
            lg_sb = lgs.tile([ARG_SHARD, CTX_PER], f32)

            for off, ln in CHUNKS:
                cs = slice(off, off + ln)
                ctt = ctp.tile([128, KCH, ln], mm_dt, tag="ctt")
                base = off * D_MODEL
                src = ct[base : base + ln * D_MODEL].rearrange(
                    "(p k c) -> p k c", p=128, k=KCH
                )
                nc.sync.dma_start(ctt[:], src)

                kt_ps = ktp.tile([KEY_DIM, ln], f32, tag="ktps")
                for k in range(KCH):
                    nc.tensor.matmul(
                        kt_ps[:],
                        wt[:, k, :],
                        ctt[:, k, :],
                        start=(k == 0),
                        stop=(k == KCH - 1),
                    )
                # PSUM -> SBUF (+ bias; b is per-partition dk here)
                if with_bias:
                    nc.vector.tensor_scalar_add(kt_sb[:, cs], kt_ps[:], bt[:])
                else:
                    nc.vector.tensor_copy(kt_sb[:, cs], kt_ps[:])

                s = off // CTX_PER
                lg_ps = lgp.tile([ARGS_PER, ln], f32, tag="lgps")
                nc.tensor.matmul(
                    lg_ps[:],
                    att[:, s * ARGS_PER : (s + 1) * ARGS_PER],
                    kt_sb[:, cs],
                    start=True,
                    stop=True,
                )
                oslice = (
                    slice(s * ARGS_PER, (s + 1) * ARGS_PER),
                    slice(off - s * CTX_PER, off - s * CTX_PER + ln),
                )
                nc.scalar.activation(
                    lg_sb[oslice], lg_ps[:], mybir.ActivationFunctionType.Copy
                )
                nc.scalar.dma_start(out[oslice], lg_sb[oslice])
    nc.finalize()
    return nc


def _get_nc(mm_dtype_name: str, with_bias: bool):
    key = (mm_dtype_name, with_bias)
    if key not in _BUILT:
        _BUILT[key] = _build_nc(mm_dtype_name, with_bias)
    return _BUILT[key]


def _pack_ct(ct_shard_t: np.ndarray) -> np.ndarray:
    """[512, 2048] C^T -> concat over chunks of [128, KCH, L] blocks."""
    parts = []
    for off, ln in CHUNKS:
        blk = ct_shard_t[:, off : off + ln].reshape(KCH, 128, ln).transpose(1, 0, 2)
        parts.append(blk.reshape(-1))
    return np.ascontiguousarray(np.concatenate(parts))


def _uniform_structure(bs, arg_ids, ctx_ids):
    if bs != BS or arg_ids.shape[0] != BS * ARGS_PER or ctx_ids.shape[0] != BS * CTX_PER:
        return False
    if not np.array_equal(np.asarray(arg_ids), np.repeat(np.arange(BS, dtype=np.int32), ARGS_PER)):
        return False
    if not np.array_equal(np.asarray(ctx_ids), np.repeat(np.arange(BS, dtype=np.int32), CTX_PER)):
        return False
    return True


def _reference_host(bs, arg_ids, ctx_ids, arg_values, ctx_values, W, b):
    """Numpy mirror of the oracle — correctness fallback for non-uniform ids."""
    n_args = arg_ids.shape[0]
    n_ctx = ctx_ids.shape[0]
    P = n_args * (n_ctx // bs)
    ctx_lens = np.bincount(ctx_ids, minlength=bs)
    arg_ctx_lens = ctx_lens[arg_ids]
    arg_ends = np.cumsum(arg_ctx_lens)
    arg_starts = arg_ends - arg_ctx_lens
    pos = np.arange(P, dtype=arg_ends.dtype)
    rows = np.searchsorted(arg_ends, pos, side="right")
    rows_c = np.clip(rows, 0, n_args - 1)
    offs = pos - arg_starts[rows_c]
    ctx_starts = np.cumsum(ctx_lens) - ctx_lens
    cols = ctx_starts[arg_ids[rows_c]] + offs
    cols = np.clip(cols, 0, n_ctx - 1)
    keys_all = ctx_values @ W + b
    logits = np.einsum(
        "pd,pd->p", arg_values[rows_c], keys_all[cols], optimize=True
    ).astype(np.float32)
    return rows.astype(np.int32), logits


LAST_EXEC_NS = None


def _install_ntff_hook():
    """Test-only: register the NTFF profile hook if the image lacks it."""
    import sys, types
    try:
        from antenv.axon_hooks import get_axon_ntff_profile_hook  # noqa: F401
        return
    except ImportError:
        pass
    import antenv
    from trn_agent_boot.trn_boot import _ntff_profile_via_ctypes

    hooks_mod = types.ModuleType("antenv.axon_hooks")
    _hook = _ntff_profile_via_ctypes("/opt/axon/libaxon_pjrt.so")
    hooks_mod.get_axon_ntff_profile_hook = lambda: _hook
    hooks_mod.set_axon_ntff_profile_hook = lambda h: None
    sys.modules["antenv.axon_hooks"] = hooks_mod
    antenv.axon_hooks = hooks_mod


def kernel(bs, arg_ids, ctx_ids, arg_values, ctx_values, W, b,
           _mm_dtype="float32r", _profile=False):
    from concourse.bass_utils import run_bass_kernel_spmd

    bs = int(np.asarray(bs))
    arg_values = np.asarray(arg_values, dtype=np.float32)
    ctx_values = np.asarray(ctx_values, dtype=np.float32)
    W = np.asarray(W, dtype=np.float32)
    b = np.asarray(b, dtype=np.float32)

    if not _uniform_structure(bs, arg_ids, ctx_ids):
        return _reference_host(
            bs, np.asarray(arg_ids), np.asarray(ctx_ids), arg_values, ctx_values, W, b
        )

    with_bias = bool(np.any(b != 0.0))
    nc = _get_nc(_mm_dtype, with_bias)

    w_arr = np.ascontiguousarray(
        W.reshape(KCH, 128, KEY_DIM).transpose(1, 0, 2)
    )
    b_arr = np.ascontiguousarray(b.reshape(KEY_DIM, 1))
    in_maps = []
    for c in range(N_CORES):
        ct_c = _pack_ct(
            np.ascontiguousarray(ctx_values[c * CTX_SHARD : (c + 1) * CTX_SHARD].T)
        )
        at_c = np.ascontiguousarray(
            arg_values[c * ARG_SHARD : (c + 1) * ARG_SHARD].T
        )
        m = {"ct": ct_c, "at": at_c, "w": w_arr}
        if with_bias:
            m["b"] = b_arr
        in_maps.append(m)

    kwargs = {}
    if _profile:
        _install_ntff_hook()
        kwargs["trace"] = True
    res = run_bass_kernel_spmd(nc, in_maps, core_ids=list(range(N_CORES)), **kwargs)
    global LAST_EXEC_NS
    LAST_EXEC_NS = res.exec_time_ns
    logits = np.concatenate(
        [np.asarray(res.results[c]["out"]).reshape(-1) for c in range(N_CORES)]
    )
    rows = np.repeat(np.arange(BS * ARGS_PER, dtype=np.int32), CTX_PER)
    return rows, logits


# revision 6
# speedup vs baseline: 1.0638x; 1.0536x over previous
"""Trainium2 Bass kernel for nn_ArgumentLocalLogits.

Math (uniform segments, BS=16, CTX_PER=1024, ARGS_PER=32):
  keys   = ctx_values @ W + b                    [n_ctx, 128]
  logits[1024*a + j] = dot(arg_values[a], keys[1024*seg(a) + j])
  rows[p] = p // 1024

Sharding: 2 proof states (segments) per core across 8 cores. Each core
gets its ctx shard pre-transposed/packed on host (d_model on partitions)
so the on-chip work is pure matmul:
  K^T[dk, ctx] = sum_k W_k^T @ CT_k  (+b)        PSUM-accumulated over 4
                                                 d_model chunks of 128
  logits_s[arg, ctx] = (A_s^T)^T @ K_s^T         per segment
"""

import numpy as np

BS = 16
CTX_PER = 1024
ARGS_PER = 32
KEY_DIM = 128
D_MODEL = 512
N_CORES = 8
SEG_PER_CORE = BS // N_CORES          # 2
CTX_SHARD = SEG_PER_CORE * CTX_PER    # 2048
ARG_SHARD = SEG_PER_CORE * ARGS_PER   # 64
KCH = D_MODEL // 128                  # 4 contraction chunks

# ctx pipeline chunks (must not cross the segment boundary at 1024);
# smaller tail chunks shrink the post-DMA critical path
CHUNKS = [(0, 512), (512, 512), (1024, 512), (1536, 256), (1792, 256)]

_BUILT = {}


def _build_nc(mm_dtype_name: str, with_bias: bool):
    import concourse.tile as tile
    from concourse import bacc, mybir

    mm_dt = getattr(mybir.dt, mm_dtype_name)
    f32 = mybir.dt.float32

    nc = bacc.Bacc(None, target_bir_lowering=False, enable_partition_id=False)
    # ct is packed on host as concat over chunks of [128, KCH, L] blocks
    ct = nc.dram_tensor("ct", [D_MODEL * CTX_SHARD], mm_dt, kind="ExternalInput")
    # wa packs W (as [128, KCH*128]) then A^T (as [128, 64]) column-wise
    wa = nc.dram_tensor("wa", [128, KCH * KEY_DIM + ARG_SHARD], mm_dt, kind="ExternalInput")
    if with_bias:
        b = nc.dram_tensor("b", [KEY_DIM, 1], f32, kind="ExternalInput")
    out = nc.dram_tensor("out", [ARG_SHARD, CTX_PER], f32, kind="ExternalOutput")

    with tile.TileContext(nc) as tc:
        with (
            tc.tile_pool(name="consts", bufs=1) as consts,
            tc.tile_pool(name="ctp", bufs=len(CHUNKS)) as ctp,
            tc.tile_pool(name="kts", bufs=1) as kts,
            tc.tile_pool(name="lgs", bufs=1) as lgs,
            tc.tile_pool(name="ktp", bufs=3, space="PSUM") as ktp,
            tc.tile_pool(name="lgp", bufs=2, space="PSUM") as lgp,
        ):
            wa_t = consts.tile([128, KCH * KEY_DIM + ARG_SHARD], mm_dt)
            nc.sync.dma_start(wa_t[:], wa[:])
            if with_bias:
                bt = consts.tile([KEY_DIM, 1], f32)
                nc.sync.dma_start(bt[:], b[:])

            kt_sb = kts.tile([KEY_DIM, CTX_SHARD], mm_dt)
            lg_sb = lgs.tile([ARG_SHARD, CTX_PER], f32)

            for off, ln in CHUNKS:
                cs = slice(off, off + ln)
                ctt = ctp.tile([128, KCH, ln], mm_dt, tag="ctt")
                base = off * D_MODEL
                src = ct[base : base + ln * D_MODEL].rearrange(
                    "(p k c) -> p k c", p=128, k=KCH
                )
                nc.sync.dma_start(ctt[:], src)

                kt_ps = ktp.tile([KEY_DIM, ln], f32, tag="ktps")
                for k in range(KCH):
                    nc.tensor.matmul(
                        kt_ps[:],
                        wa_t[:, k * KEY_DIM : (k + 1) * KEY_DIM],
                        ctt[:, k, :],
                        start=(k == 0),
                        stop=(k == KCH - 1),
                    )
                # PSUM -> SBUF (+ bias; b is per-partition dk here)
                if with_bias:
                    nc.vector.tensor_scalar_add(kt_sb[:, cs], kt_ps[:], bt[:])
                else:
                    nc.vector.tensor_copy(kt_sb[:, cs], kt_ps[:])

                s = off // CTX_PER
                lg_ps = lgp.tile([ARGS_PER, ln], f32, tag="lgps")
                nc.tensor.matmul(
                    lg_ps[:],
                    wa_t[:, KCH * KEY_DIM + s * ARGS_PER : KCH * KEY_DIM + (s + 1) * ARGS_PER],
                    kt_sb[:, cs],
                    start=True,
                    stop=True,
                )
                oslice = (
                    slice(s * ARGS_PER, (s + 1) * ARGS_PER),
                    slice(off - s * CTX_PER, off - s * CTX_PER + ln),
                )
                nc.scalar.activation(
                    lg_sb[oslice], lg_ps[:], mybir.ActivationFunctionType.Copy
                )
                nc.scalar.dma_start(out[oslice], lg_sb[oslice])
    nc.finalize()
    return nc


def _get_nc(mm_dtype_name: str, with_bias: bool):
    key = (mm_dtype_name, with_bias)
    if key not in _BUILT:
        _BUILT[key] = _build_nc(mm_dtype_name, with_bias)
    return _BUILT[key]


def _pack_ct(ct_shard_t: np.ndarray) -> np.ndarray:
    """[512, 2048] C^T -> concat over chunks of [128, KCH, L] blocks."""
    parts = []
    for off, ln in CHUNKS:
        blk = ct_shard_t[:, off : off + ln].reshape(KCH, 128, ln).transpose(1, 0, 2)
        parts.append(blk.reshape(-1))
    return np.ascontiguousarray(np.concatenate(parts))


def _uniform_structure(bs, arg_ids, ctx_ids):
    if bs != BS or arg_ids.shape[0] != BS * ARGS_PER or ctx_ids.shape[0] != BS * CTX_PER:
        return False
    if not np.array_equal(np.asarray(arg_ids), np.repeat(np.arange(BS, dtype=np.int32), ARGS_PER)):
        return False
    if not np.array_equal(np.asarray(ctx_ids), np.repeat(np.arange(BS, dtype=np.int32), CTX_PER)):
        return False
    return True


def _reference_host(bs, arg_ids, ctx_ids, arg_values, ctx_values, W, b):
    """Numpy mirror of the oracle — correctness fallback for non-uniform ids."""
    n_args = arg_ids.shape[0]
    n_ctx = ctx_ids.shape[0]
    P = n_args * (n_ctx // bs)
    ctx_lens = np.bincount(ctx_ids, minlength=bs)
    arg_ctx_lens = ctx_lens[arg_ids]
    arg_ends = np.cumsum(arg_ctx_lens)
    arg_starts = arg_ends - arg_ctx_lens
    pos = np.arange(P, dtype=arg_ends.dtype)
    rows = np.searchsorted(arg_ends, pos, side="right")
    rows_c = np.clip(rows, 0, n_args - 1)
    offs = pos - arg_starts[rows_c]
    ctx_starts = np.cumsum(ctx_lens) - ctx_lens
    cols = ctx_starts[arg_ids[rows_c]] + offs
    cols = np.clip(cols, 0, n_ctx - 1)
    keys_all = ctx_values @ W + b
    logits = np.einsum(
        "pd,pd->p", arg_values[rows_c], keys_all[cols], optimize=True
    ).astype(np.float32)
    return rows.astype(np.int32), logits


LAST_EXEC_NS = None


def _install_ntff_hook():
    """Test-only: register the NTFF profile hook if the image lacks it."""
    import sys, types
    try:
        from antenv.axon_hooks import get_axon_ntff_profile_hook  # noqa: F401
        return
    except ImportError:
        pass
    import antenv
    from trn_agent_boot.trn_boot import _ntff_profile_via_ctypes

    hooks_mod = types.ModuleType("antenv.axon_hooks")
    _hook = _ntff_profile_via_ctypes("/opt/axon/libaxon_pjrt.so")
    hooks_mod.get_axon_ntff_profile_hook = lambda: _hook
    hooks_mod.set_axon_ntff_profile_hook = lambda h: None
    sys.modules["antenv.axon_hooks"] = hooks_mod
    antenv.axon_hooks = hooks_mod


def kernel(bs, arg_ids, ctx_ids, arg_values, ctx_values, W, b,
           _mm_dtype="float32r", _profile=False):
    from concourse.bass_utils import run_bass_kernel_spmd

    bs = int(np.asarray(bs))
    arg_values = np.asarray(arg_values, dtype=np.float32)
    ctx_values = np.asarray(ctx_values, dtype=np.float32)
    W = np.asarray(W, dtype=np.float32)
    b = np.asarray(b, dtype=np.float32)

    if not _uniform_structure(bs, arg_ids, ctx_ids):
        return _reference_host(
            bs, np.asarray(arg_ids), np.asarray(ctx_ids), arg_values, ctx_values, W, b
        )

    with_bias = bool(np.any(b != 0.0))
    nc = _get_nc(_mm_dtype, with_bias)

    w_arr = W.reshape(KCH, 128, KEY_DIM).transpose(1, 0, 2).reshape(128, KCH * KEY_DIM)
    b_arr = np.ascontiguousarray(b.reshape(KEY_DIM, 1))
    in_maps = []
    for c in range(N_CORES):
        ct_c = _pack_ct(
            np.ascontiguousarray(ctx_values[c * CTX_SHARD : (c + 1) * CTX_SHARD].T)
        )
        at_c = arg_values[c * ARG_SHARD : (c + 1) * ARG_SHARD].T
        wa_c = np.ascontiguousarray(np.concatenate([w_arr, at_c], axis=1))
        m = {"ct": ct_c, "wa": wa_c}
        if with_bias:
            m["b"] = b_arr
        in_maps.append(m)

    kwargs = {}
    if _profile:
        _install_ntff_hook()
        kwargs["trace"] = True
    res = run_bass_kernel_spmd(nc, in_maps, core_ids=list(range(N_CORES)), **kwargs)
    global LAST_EXEC_NS
    LAST_EXEC_NS = res.exec_time_ns
    logits = np.concatenate(
        [np.asarray(res.results[c]["out"]).reshape(-1) for c in range(N_CORES)]
    )
    rows = np.repeat(np.arange(BS * ARGS_PER, dtype=np.int32), CTX_PER)
    return rows, logits


# revision 7
# speedup vs baseline: 1.3704x; 1.2882x over previous
"""Trainium2 Bass kernel for nn_ArgumentLocalLogits.

Math (uniform segments, BS=16, CTX_PER=1024, ARGS_PER=32):
  keys   = ctx_values @ W + b                    [n_ctx, 128]
  logits[1024*a + j] = dot(arg_values[a], keys[1024*seg(a) + j])
  rows[p] = p // 1024

Sharding: 2 proof states (segments) per core across 8 cores. Each core
gets its ctx shard pre-transposed/packed on host (d_model on partitions)
so the on-chip work is pure matmul:
  K^T[dk, ctx] = sum_k W_k^T @ CT_k  (+b)        PSUM-accumulated over 4
                                                 d_model chunks of 128
  logits_s[arg, ctx] = (A_s^T)^T @ K_s^T         per segment
"""

import numpy as np

BS = 16
CTX_PER = 1024
ARGS_PER = 32
KEY_DIM = 128
D_MODEL = 512
N_CORES = 8
SEG_PER_CORE = BS // N_CORES          # 2
CTX_SHARD = SEG_PER_CORE * CTX_PER    # 2048
ARG_SHARD = SEG_PER_CORE * ARGS_PER   # 64
KCH = D_MODEL // 128                  # 4 contraction chunks

# ctx pipeline chunks (must not cross the segment boundary at 1024);
# smaller tail chunks shrink the post-DMA critical path
CHUNKS = [(0, 512), (512, 512), (1024, 512), (1536, 256), (1792, 256)]

_BUILT = {}


def _build_nc(mm_dtype_name: str, with_bias: bool):
    import concourse.tile as tile
    from concourse import bacc, mybir

    mm_dt = getattr(mybir.dt, mm_dtype_name)
    f32 = mybir.dt.float32

    nc = bacc.Bacc(None, target_bir_lowering=False, enable_partition_id=False)
    # ct is packed on host as concat over chunks of [128, KCH, L] blocks
    ct = nc.dram_tensor("ct", [D_MODEL * CTX_SHARD], mm_dt, kind="ExternalInput")
    # wa packs W (as [128, KCH*128]) then A^T (as [128, 64]) column-wise
    wa = nc.dram_tensor("wa", [128, KCH * KEY_DIM + ARG_SHARD], mm_dt, kind="ExternalInput")
    if with_bias:
        b = nc.dram_tensor("b", [KEY_DIM, 1], f32, kind="ExternalInput")
    out = nc.dram_tensor("out", [ARG_SHARD, CTX_PER], f32, kind="ExternalOutput")

    with tile.TileContext(nc) as tc:
        with (
            tc.tile_pool(name="consts", bufs=1) as consts,
            tc.tile_pool(name="ctp", bufs=len(CHUNKS)) as ctp,
            tc.tile_pool(name="kts", bufs=1) as kts,
            tc.tile_pool(name="lgs", bufs=1) as lgs,
            tc.tile_pool(name="ktp", bufs=3, space="PSUM") as ktp,
            tc.tile_pool(name="lgp", bufs=2, space="PSUM") as lgp,
        ):
            wa_t = consts.tile([128, KCH * KEY_DIM + ARG_SHARD], mm_dt)
            nc.sync.dma_start(wa_t[:], wa[:])
            if with_bias:
                bt = consts.tile([KEY_DIM, 1], f32)
                nc.sync.dma_start(bt[:], b[:])

            kt_sb = kts.tile([KEY_DIM, CTX_SHARD], mm_dt)
            lg_sb = lgs.tile([ARG_SHARD, CTX_PER], f32)

            for off, ln in CHUNKS:
                cs = slice(off, off + ln)
                ctt = ctp.tile([128, KCH, ln], mm_dt, tag="ctt")
                base = off * D_MODEL
                src = ct[base : base + ln * D_MODEL].rearrange(
                    "(p k c) -> p k c", p=128, k=KCH
                )
                nc.sync.dma_start(ctt[:], src)

                kt_ps = ktp.tile([KEY_DIM, ln], f32, tag="ktps")
                for k in range(KCH):
                    nc.tensor.matmul(
                        kt_ps[:],
                        wa_t[:, k * KEY_DIM : (k + 1) * KEY_DIM],
                        ctt[:, k, :],
                        start=(k == 0),
                        stop=(k == KCH - 1),
                    )
                # PSUM -> SBUF (+ bias; b is per-partition dk here)
                if with_bias:
                    nc.vector.tensor_scalar_add(kt_sb[:, cs], kt_ps[:], bt[:])
                else:
                    nc.vector.tensor_copy(kt_sb[:, cs], kt_ps[:])

                s = off // CTX_PER
                lg_ps = lgp.tile([ARGS_PER, ln], f32, tag="lgps")
                nc.tensor.matmul(
                    lg_ps[:],
                    wa_t[:, KCH * KEY_DIM + s * ARGS_PER : KCH * KEY_DIM + (s + 1) * ARGS_PER],
                    kt_sb[:, cs],
                    start=True,
                    stop=True,
                )
                oslice = (
                    slice(s * ARGS_PER, (s + 1) * ARGS_PER),
                    slice(off - s * CTX_PER, off - s * CTX_PER + ln),
                )
                nc.scalar.activation(
                    lg_sb[oslice], lg_ps[:], mybir.ActivationFunctionType.Copy
                )
                nc.scalar.dma_start(out[oslice], lg_sb[oslice])
    nc.finalize()
    return nc


def _get_nc(mm_dtype_name: str, with_bias: bool):
    key = (mm_dtype_name, with_bias)
    if key not in _BUILT:
        _BUILT[key] = _build_nc(mm_dtype_name, with_bias)
    return _BUILT[key]


def _pack_ct(ct_shard_t: np.ndarray) -> np.ndarray:
    """[512, 2048] C^T -> concat over chunks of [128, KCH, L] blocks."""
    parts = []
    for off, ln in CHUNKS:
        blk = ct_shard_t[:, off : off + ln].reshape(KCH, 128, ln).transpose(1, 0, 2)
        parts.append(blk.reshape(-1))
    return np.ascontiguousarray(np.concatenate(parts))


def _uniform_structure(bs, arg_ids, ctx_ids):
    if bs != BS or arg_ids.shape[0] != BS * ARGS_PER or ctx_ids.shape[0] != BS * CTX_PER:
        return False
    if not np.array_equal(np.asarray(arg_ids), np.repeat(np.arange(BS, dtype=np.int32), ARGS_PER)):
        return False
    if not np.array_equal(np.asarray(ctx_ids), np.repeat(np.arange(BS, dtype=np.int32), CTX_PER)):
        return False
    return True


def _reference_host(bs, arg_ids, ctx_ids, arg_values, ctx_values, W, b):
    """Numpy mirror of the oracle — correctness fallback for non-uniform ids."""
    n_args = arg_ids.shape[0]
    n_ctx = ctx_ids.shape[0]
    P = n_args * (n_ctx // bs)
    ctx_lens = np.bincount(ctx_ids, minlength=bs)
    arg_ctx_lens = ctx_lens[arg_ids]
    arg_ends = np.cumsum(arg_ctx_lens)
    arg_starts = arg_ends - arg_ctx_lens
    pos = np.arange(P, dtype=arg_ends.dtype)
    rows = np.searchsorted(arg_ends, pos, side="right")
    rows_c = np.clip(rows, 0, n_args - 1)
    offs = pos - arg_starts[rows_c]
    ctx_starts = np.cumsum(ctx_lens) - ctx_lens
    cols = ctx_starts[arg_ids[rows_c]] + offs
    cols = np.clip(cols, 0, n_ctx - 1)
    keys_all = ctx_values @ W + b
    logits = np.einsum(
        "pd,pd->p", arg_values[rows_c], keys_all[cols], optimize=True
    ).astype(np.float32)
    return rows.astype(np.int32), logits


LAST_EXEC_NS = None


def _install_ntff_hook():
    """Test-only: register the NTFF profile hook if the image lacks it."""
    import sys, types
    try:
        from antenv.axon_hooks import get_axon_ntff_profile_hook  # noqa: F401
        return
    except ImportError:
        pass
    import antenv
    from trn_agent_boot.trn_boot import _ntff_profile_via_ctypes

    hooks_mod = types.ModuleType("antenv.axon_hooks")
    _hook = _ntff_profile_via_ctypes("/opt/axon/libaxon_pjrt.so")
    hooks_mod.get_axon_ntff_profile_hook = lambda: _hook
    hooks_mod.set_axon_ntff_profile_hook = lambda h: None
    sys.modules["antenv.axon_hooks"] = hooks_mod
    antenv.axon_hooks = hooks_mod


def kernel(bs, arg_ids, ctx_ids, arg_values, ctx_values, W, b,
           _mm_dtype="float32r", _profile=False):
    from concourse.bass_utils import run_bass_kernel_spmd

    bs = int(np.asarray(bs))
    arg_values = np.asarray(arg_values, dtype=np.float32)
    ctx_values = np.asarray(ctx_values, dtype=np.float32)
    W = np.asarray(W, dtype=np.float32)
    b = np.asarray(b, dtype=np.float32)

    if not _uniform_structure(bs, arg_ids, ctx_ids):
        return _reference_host(
            bs, np.asarray(arg_ids), np.asarray(ctx_ids), arg_values, ctx_values, W, b
        )

    with_bias = bool(np.any(b != 0.0))
    nc = _get_nc(_mm_dtype, with_bias)

    host_dt = {"float32r": np.float32, "float32": np.float32,
               "float16": np.float16}[_mm_dtype]
    w_arr = W.reshape(KCH, 128, KEY_DIM).transpose(1, 0, 2).reshape(128, KCH * KEY_DIM)
    b_arr = np.ascontiguousarray(b.reshape(KEY_DIM, 1))
    in_maps = []
    for c in range(N_CORES):
        ct_c = _pack_ct(
            np.ascontiguousarray(ctx_values[c * CTX_SHARD : (c + 1) * CTX_SHARD].T)
        ).astype(host_dt)
        at_c = arg_values[c * ARG_SHARD : (c + 1) * ARG_SHARD].T
        wa_c = np.ascontiguousarray(np.concatenate([w_arr, at_c], axis=1)).astype(host_dt)
        m = {"ct": ct_c, "wa": wa_c}
        if with_bias:
            m["b"] = b_arr
        in_maps.append(m)

    kwargs = {}
    if _profile:
        _install_ntff_hook()
        kwargs["trace"] = True
    res = run_bass_kernel_spmd(nc, in_maps, core_ids=list(range(N_CORES)), **kwargs)
    global LAST_EXEC_NS
    LAST_EXEC_NS = res.exec_time_ns
    logits = np.concatenate(
        [np.asarray(res.results[c]["out"]).reshape(-1) for c in range(N_CORES)]
    )
    rows = np.repeat(np.arange(BS * ARGS_PER, dtype=np.int32), CTX_PER)
    return rows, logits
